# revision 1
# baseline (speedup 1.0000x reference)
"""Trainium2 Bass kernel for MultiLinearAttention (causal linear attention).

Reference computation (per head h, feature map phi(u) = elu(u)+1):
    q = phi(x_h @ Wq_h), k = phi(x_h @ Wk_h), v = x_h @ Wv_h
    y_t = (q_t . sum_{s<=t} k_s v_s^T) / (q_t . sum_{s<=t} k_s + eps)
    out = concat_h(y_h) @ Wp

Sharding: 16 heads / 8 cores = 2 heads per core, all 4 batches per core.
Wp is folded per-head into the v projection (W'_h = Wv_h @ Wp_h), so each
core produces a partial [B, S, 64] output summed on the host (the unshard
step for this head/output-partial sharding).

Device algorithm: chunked causal linear attention, chunk C=128:
    A^T = K_chunk Q_chunk^T (per head), masked to s<=t
    num = A_m^T V'aug + Q^T S_aug   (aug col of V' is ones -> den)
    S_aug += K_chunk^T V'aug
phi is computed as  max(u+1, min(exp(u), 1))  == elu(u)+1, with the +1
coming from presetting PSUM to 1 via a K=1 ones matmul.

Two batches are processed per instruction ("pair batching") to amortize
fixed per-instruction engine costs.
"""

import os
import sys

import numpy as np

for _p in ("/root/.axon_site/_ro/trn_rl_repo", "/opt/trn_rl_repo", "/opt/pypackages"):
    if os.path.isdir(_p) and _p not in sys.path:
        sys.path.append(_p)

import ml_dtypes

B, S, D = 4, 4096, 1024
H, HD, O = 16, 64, 64
C = 128                  # chunk length
NCORE = 8
HPC = H // NCORE         # heads per core
NCHUNK = S // C

USE_BF16 = True

_CACHE = {}


def _build_program(nchunk=NCHUNK, stage=99):
    import concourse.mybir as mybir
    from concourse import bacc
    from concourse.tile import TileContext

    fp32 = mybir.dt.float32
    cdt = mybir.dt.bfloat16 if USE_BF16 else fp32
    Alu = mybir.AluOpType
    Act = mybir.ActivationFunctionType

    nc = bacc.Bacc()
    xT_h = nc.declare_dram_parameter("xT", [B, 128, S], cdt, isOutput=False)
    wq_h = nc.declare_dram_parameter("wq", [128, 128], cdt, isOutput=False)
    wk_h = nc.declare_dram_parameter("wk", [128, 128], cdt, isOutput=False)
    wv_h = nc.declare_dram_parameter("wv", [128, 128], cdt, isOutput=False)
    mask_h = nc.declare_dram_parameter("mask2", [128, 512], cdt, isOutput=False)
    ident_h = nc.declare_dram_parameter("ident", [128, 128], cdt, isOutput=False)
    ones_h = nc.declare_dram_parameter("ones", [1, 512], cdt, isOutput=False)
    zer_h = nc.declare_dram_parameter("zer", [1, 512], cdt, isOutput=False)
    out_h = nc.declare_dram_parameter("out", [B, S, O], fp32, isOutput=True)

    with TileContext(nc) as tc:
        with (
            tc.tile_pool(name="consts", bufs=1) as consts,
            tc.tile_pool(name="work", bufs=4) as work,
            tc.tile_pool(name="st_sb", bufs=3) as st_sb,
            tc.tile_pool(name="pu", bufs=1, space="PSUM") as pu,
            tc.tile_pool(name="pa", bufs=1, space="PSUM") as pa,
            tc.tile_pool(name="pvk", bufs=1, space="PSUM") as pvk,
            tc.tile_pool(name="pkn", bufs=1, space="PSUM") as pkn,
            tc.tile_pool(name="pnum", bufs=1, space="PSUM") as pnum,
            tc.tile_pool(name="pst", bufs=1, space="PSUM") as pst,
        ):
            # ---- constants into SBUF ----
            neg1 = consts.tile([128, 1], fp32)
            nc.gpsimd.memset(neg1, -1.0)
            wq = consts.tile([128, 128], cdt)
            wk = consts.tile([128, 128], cdt)
            wv = consts.tile([128, 128], cdt)
            mask2 = consts.tile([128, 512], cdt)
            ident = consts.tile([128, 128], cdt)
            ones = consts.tile([1, 512], cdt)
            zer = consts.tile([1, 512], cdt)
            nc.sync.dma_start(wq, wq_h[:, :])
            nc.sync.dma_start(wk, wk_h[:, :])
            nc.sync.dma_start(wv, wv_h[:, :])
            nc.sync.dma_start(mask2, mask_h[:, :])
            nc.sync.dma_start(ident, ident_h[:, :])
            nc.sync.dma_start(ones, ones_h[:, :])
            nc.sync.dma_start(zer, zer_h[:, :])

            xsb = []
            for b in range(B):
                xb = consts.tile([128, S], cdt, name=f"xsb{b}")
                nc.sync.dma_start(xb, xT_h[b])
                xsb.append(xb)

            # persistent per-pair state PSUM: [S'(b_even) | S'(b_odd)], each
            # [128, 130] with head0 block [0:64, 0:65], head1 [64:128, 65:130]
            st_ps = [
                pst.tile([128, 260], fp32, name="stA"),
                pst.tile([128, 260], fp32, name="stB"),
            ]
            # One start=True zero-write owns each state bank; all later state
            # matmuls accumulate with start=False. (start=True marks the whole
            # 2KB PSUM zero-region pending, so it must appear exactly once.)
            for stp in st_ps:
                nc.tensor.matmul(stp, ones[:, 0:128], zer[:, 0:260],
                                 start=True, stop=False, skip_group_check=True)

            s01_prev = [None, None]

            for i in range(nchunk):
                sl = slice(i * C, (i + 1) * C)
                for pr in range(2):
                    b0, b1 = 2 * pr, 2 * pr + 1
                    stp = st_ps[pr]

                    # ---------------- PE: projections ----------------
                    # u layout: [q(b0) | k(b0) | q(b1) | k(b1)] each [128,128]
                    u = pu.tile([128, 512], fp32, name="u")
                    # preset PSUM to 1.0 so u holds w = proj + 1
                    nc.tensor.matmul(u, ones[:, 0:128], ones, start=True,
                                     stop=False, skip_group_check=True)
                    for j, xb in enumerate((xsb[b0], xsb[b1])):
                        nc.tensor.matmul(u[:, 256 * j:256 * j + 128], wq,
                                         xb[:, sl], start=False, stop=True,
                                         skip_group_check=True)
                    for j, xb in enumerate((xsb[b0], xsb[b1])):
                        nc.tensor.matmul(u[:, 256 * j + 128:256 * j + 256], wk,
                                         xb[:, sl], start=False, stop=True,
                                         skip_group_check=True)
                    # vk psum: [v'(b0) | v'(b1)]
                    vk = pvk.tile([128, 256], fp32, name="vk")
                    for j, xb in enumerate((xsb[b0], xsb[b1])):
                        nc.tensor.matmul(vk[:, 128 * j:128 * (j + 1)],
                                         xb[:, sl], wv, start=True, stop=True)

                    # ---------------- phi ----------------
                    # e = exp(w - 1) = exp(u);  phi = max(w, min(e, 1))
                    e2 = work.tile([128, 512], cdt, name="e2")
                    nc.scalar.activation(e2, u, Act.Exp, bias=neg1)
                    phi2 = work.tile([128, 512], cdt, name="phi2")
                    nc.vector.scalar_tensor_tensor(
                        phi2, e2, 1.0, u, Alu.min, Alu.max)

                    if stage < 1:
                        continue
                    # ---------------- A^T = K Q^T per (b, h) ----------------
                    # One PSUM bank per head: all matmuls writing a given bank
                    # must read operands from the same base partition (HW).
                    ah = [pa.tile([128, 256], fp32, name="a_h0"),
                          pa.tile([128, 256], fp32, name="a_h1")]
                    for j in range(2):
                        qq = phi2[:, 256 * j:256 * j + 128]
                        kk = phi2[:, 256 * j + 128:256 * j + 256]
                        for h in range(2):
                            es = slice(64 * h, 64 * (h + 1))
                            nc.tensor.matmul(
                                ah[h][:, 128 * j:128 * (j + 1)],
                                kk[es, :], qq[es, :], start=True, stop=True)

                    if stage < 2:
                        continue
                    # knat via PE transpose of phi(k) into bf16 PSUM
                    knp = pkn.tile([128, 256], cdt, name="knp")
                    for j in range(2):
                        nc.tensor.transpose(
                            knp[:, 128 * j:128 * (j + 1)],
                            phi2[:, 256 * j + 128:256 * j + 256], ident)

                    # masked A -> SBUF; layout [b0h0 | b1h0 | b0h1 | b1h1]
                    am2 = work.tile([128, 512], cdt, name="am2")
                    nc.vector.tensor_tensor(am2[:, 0:256], ah[0], mask2[:, 0:256],
                                            Alu.mult)
                    nc.vector.tensor_tensor(am2[:, 256:512], ah[1],
                                            mask2[:, 256:512], Alu.mult)

                    # V'aug: [v0 | 1 | v1 | 1] per b  (130 cols per b)
                    vaug = work.tile([128, 260], cdt, name="vaug")
                    vsrc = vk.rearrange("p (g c) -> p g c", c=64)
                    vdst = vaug.rearrange("p (g c) -> p g c", c=65)[:, :, 0:64]
                    nc.scalar.copy(vdst, vsrc)
                    vones = vaug.rearrange("p (g c) -> p g c", c=65)[:, :, 64:65]
                    nc.gpsimd.memset(vones, 1.0)

                    knat2 = work.tile([128, 256], cdt, name="knat2")
                    nc.vector.tensor_copy(knat2, knp)

                    if stage < 3:
                        continue
                    # ---------------- num = A_m^T Vaug + Q^T S ----------------
                    num = pnum.tile([128, 260], fp32, name="num")
                    nc.tensor.matmul(num, ones[:, 0:128], zer[:, 0:260],
                                     start=True, stop=False,
                                     skip_group_check=True)
                    # For b1, head blocks are stored swapped (h1 first) so the
                    # h1 state block (output partition offset 64) lands at a
                    # column where its AP stays within one PSUM bank. Heads
                    # are summed at the end, so block identity is positional.
                    for j in range(2):
                        for h in range(2):
                            hp = h ^ j  # head's positional slot
                            reg = slice(130 * j + 65 * hp, 130 * j + 65 * (hp + 1))
                            va = vaug[:, 130 * j + 65 * h:130 * j + 65 * (h + 1)]
                            nc.tensor.matmul(
                                num[:, reg],
                                am2[:, 256 * h + 128 * j:256 * h + 128 * (j + 1)],
                                va, start=False, stop=False,
                                skip_group_check=True)
                        if i > 0:
                            # both heads at once: K=128 with block-diag state
                            sp = s01_prev[pr]
                            nc.tensor.matmul(
                                num[:, 130 * j:130 * (j + 1)],
                                phi2[:, 256 * j:256 * j + 128],
                                sp[:, 130 * j:130 * (j + 1)],
                                start=False, stop=True,
                                skip_group_check=True)

                    if stage < 4:
                        continue
                    # ---------------- state update (diag blocks only) --------
                    # Per-head matmuls with base-0 operands; h1 writes at
                    # output partition offset 64. Off-diag blocks stay zero so
                    # the s01 copy is directly block-diagonal.
                    for j in range(2):
                        for h in range(2):
                            hp = h ^ j
                            nc.tensor.matmul(
                                stp[64 * h:64 * (h + 1),
                                    130 * j + 65 * hp:130 * j + 65 * (hp + 1)],
                                knat2[:, 128 * j + 64 * h:128 * j + 64 * (h + 1)],
                                vaug[:, 130 * j + 65 * h:130 * j + 65 * (h + 1)],
                                start=False, stop=False,
                                skip_group_check=True)

                    if i < NCHUNK - 1:
                        s01 = st_sb.tile([128, 260], cdt, name="s01")
                        nc.scalar.copy(s01, stp)
                        s01_prev[pr] = s01

                    if stage < 5:
                        continue
                    # ---------------- y = num/den, sum heads ----------------
                    rec = work.tile([128, 4], fp32, name="rec")
                    dens = num.rearrange("p (g c) -> p g c", c=65)[:, :, 64:65]
                    nc.vector.reciprocal(rec, dens)
                    for j, b in enumerate((b0, b1)):
                        y1 = work.tile([128, 64], fp32, name=f"y1_{j}")
                        nc.vector.tensor_scalar_mul(
                            y1, num[:, 130 * j + 65:130 * j + 129],
                            rec[:, 2 * j + 1:2 * j + 2])
                        yo = work.tile([128, 64], fp32, name=f"yo_{j}")
                        nc.vector.scalar_tensor_tensor(
                            yo, num[:, 130 * j:130 * j + 64],
                            rec[:, 2 * j:2 * j + 1], y1, Alu.mult, Alu.add)
                        nc.sync.dma_start(out_h[b, sl, :], yo)

    nc.finalize()
    return nc


def _host_prep(x, Wq, Wk, Wv, Wp):
    """Shard inputs per core; returns in_maps list."""
    x = np.asarray(x, dtype=np.float32)
    Wq = np.asarray(Wq, dtype=np.float32)
    Wk = np.asarray(Wk, dtype=np.float32)
    Wv = np.asarray(Wv, dtype=np.float32)
    Wp = np.asarray(Wp, dtype=np.float32)
    ndt = ml_dtypes.bfloat16 if USE_BF16 else np.float32

    mask = np.triu(np.ones((C, C), np.float32))
    mask2 = np.tile(mask, (1, 4)).astype(ndt)          # [128, 512]
    ident = np.eye(128, dtype=np.float32).astype(ndt)
    ones = np.ones((1, 512), np.float32).astype(ndt)

    in_maps = []
    for c in range(NCORE):
        h0 = HPC * c
        xs = x[:, :, 64 * h0:64 * (h0 + HPC)]          # [B, S, 128]
        xT = np.ascontiguousarray(xs.transpose(0, 2, 1)).astype(ndt)
        wq_bd = np.zeros((128, 128), np.float32)
        wk_bd = np.zeros((128, 128), np.float32)
        wv_bd = np.zeros((128, 128), np.float32)
        for j in range(HPC):
            h = h0 + j
            sl = slice(64 * j, 64 * (j + 1))
            wq_bd[sl, sl] = Wq[h]
            wk_bd[sl, sl] = Wk[h]
            wv_bd[sl, sl] = Wv[h] @ Wp[64 * h:64 * (h + 1), :]
        in_maps.append({
            "xT": xT,
            "wq": wq_bd.astype(ndt),
            "wk": wk_bd.astype(ndt),
            "wv": wv_bd.astype(ndt),
            "mask2": mask2,
            "ident": ident,
            "ones": ones,
            "zer": np.zeros((1, 512), np.float32).astype(ndt),
        })
    return in_maps


def get_program():
    if "nc" not in _CACHE:
        _CACHE["nc"] = _build_program()
    return _CACHE["nc"]


def run_spmd(in_maps, **kwargs):
    from concourse.bass_utils import run_bass_kernel_spmd
    nc = get_program()
    return run_bass_kernel_spmd(nc, in_maps, list(range(NCORE)), **kwargs)


def kernel(x, Wq, Wk, Wv, Wp):
    in_maps = _host_prep(x, Wq, Wk, Wv, Wp)
    res = run_spmd(in_maps)
    out = np.zeros((B, S, O), np.float32)
    for c in range(NCORE):
        out += res.results[c]["out"]
    return out

